# revision 6
# baseline (speedup 1.0000x reference)
"""MoE layer kernel for Trainium2 (8 NeuronCores, SPMD via bass/Tile).

Strategy:
  - Host: gate (global-avg-pool -> Linear -> softmax -> top-2). Only the
    top-2 experts per sample contribute to the output (exp_w is zero
    elsewhere), so we compute just those: 16 (sample, expert) pairs.
  - Device: core b processes sample b with its 2 selected experts.
    out = x + sum_e (s_e * W2_e)^T gelu(W1_e^T x + b1_e)
    where s_e = topk_w[b,e] * k[b] is folded into W2 on the host.
    The b2 contribution (sum_e s_e*b2_e, a per-channel constant) is added
    on the host afterwards (it is zero for this module's init anyway).
  - Matmuls run in float32r (fp32 data, 1 cycle/row on the PE at N=512).
  - All inputs are pre-packed on the host into the exact per-partition
    SBUF layout so every DMA is 128 large contiguous descriptors, and
    DMAs are split so compute starts as soon as the first tiles land.
"""

import os
import numpy as np

P = 128
C = 512
DH = 1024
HW = 1024
CO = C // P     # 4 chunks of C on partitions
DO = DH // P    # 8 chunks of Dh on partitions
NF = 512        # matmul moving-dim tile
NH = HW // NF   # 2
E2 = 2          # experts per sample (top-k)
B = 8

MM_DTYPE = os.environ.get("MOE_MM_DTYPE", "float32r")

_NC_CACHE = {}


def _build_nc(mm_dtype_name):
    import concourse.mybir as mybir
    import concourse.tile as tile
    from concourse import bacc

    fp32 = mybir.dt.float32
    mmdt = getattr(mybir.dt, mm_dtype_name)

    nc = bacc.Bacc("TRN2", target_bir_lowering=False, debug=False, num_devices=B)

    # DRAM inputs pre-packed to per-partition layout (host does the packing)
    x_d = nc.dram_tensor("x", [P, NH, CO, NF], mmdt, kind="ExternalInput")
    w1_d = nc.dram_tensor("w1", [P, E2, DO, CO, P], mmdt, kind="ExternalInput")
    b1_d = nc.dram_tensor("b1", [P, E2, DO], fp32, kind="ExternalInput")
    w2_d = nc.dram_tensor("w2", [P, E2, DO, C], mmdt, kind="ExternalInput")
    out_d = nc.dram_tensor("out", [C, HW], fp32, kind="ExternalOutput")

    with tile.TileContext(nc) as tc:
        with (
            tc.tile_pool(name="const", bufs=1) as cpool,
            tc.tile_pool(name="psh", bufs=4, space="PSUM") as ph_pool,
            tc.tile_pool(name="psy", bufs=4, space="PSUM") as py_pool,
            tc.tile_pool(name="outp", bufs=4) as opool,
        ):
            x_sb = cpool.tile([P, NH, CO, NF], mmdt)
            w1_sb = cpool.tile([P, E2, DO, CO, P], mmdt)
            b1_sb = cpool.tile([P, E2, DO], fp32)
            w2_sb = cpool.tile([P, E2, DO, C], mmdt)
            h_sb = cpool.tile([P, E2, DO, HW], mmdt)

            # DMAs in consumption order; first matmul needs only x[half0]
            # and w1[e0,do0]. x/w1/b1 on the sync HWDGE ring, w2 on the
            # scalar ring so the two streams issue in parallel.
            nc.sync.dma_start(x_sb[:, 0], x_d.ap()[:, 0])
            nc.sync.dma_start(w1_sb[:, 0, 0], w1_d.ap()[:, 0, 0])
            nc.sync.dma_start(b1_sb[:], b1_d.ap()[:])
            for do in range(1, DO):
                nc.sync.dma_start(w1_sb[:, 0, do], w1_d.ap()[:, 0, do])
            for do in range(DO):
                nc.sync.dma_start(w1_sb[:, 1, do], w1_d.ap()[:, 1, do])
            nc.sync.dma_start(x_sb[:, 1], x_d.ap()[:, 1])
            for e in range(E2):
                for do in range(DO):
                    nc.scalar.dma_start(w2_sb[:, e, do], w2_d.ap()[:, e, do])

            # Stage A: h[e] = gelu(W1_e^T x + b1_e)   (partitions: Dh chunk)
            for half in range(NH):
                hw_sl = slice(half * NF, (half + 1) * NF)
                for e in range(E2):
                    for do in range(DO):
                        ps = ph_pool.tile([P, NF], fp32, tag="ps_h")
                        for co in range(CO):
                            nc.tensor.matmul(
                                ps[:],
                                w1_sb[:, e, do, co, :],
                                x_sb[:, half, co, :],
                                start=(co == 0),
                                stop=(co == CO - 1),
                            )
                        nc.scalar.activation(
                            h_sb[:, e, do, hw_sl],
                            ps[:],
                            mybir.ActivationFunctionType.Gelu,
                            bias=b1_sb[:, e, do:do + 1],
                            scale=1.0,
                        )

            # Stage B: out = x + sum_e (s_e W2_e)^T h_e  (partitions: C chunk)
            out_r = out_d.ap().rearrange("(o p) f -> p o f", p=P)
            for half in range(NH):
                hw_sl = slice(half * NF, (half + 1) * NF)
                for co in range(CO):
                    ps = py_pool.tile([P, NF], fp32, tag="ps_y")
                    n_acc = E2 * DO
                    i = 0
                    for e in range(E2):
                        for do in range(DO):
                            nc.tensor.matmul(
                                ps[:],
                                w2_sb[:, e, do, co * P:(co + 1) * P],
                                h_sb[:, e, do, hw_sl],
                                start=(i == 0),
                                stop=(i == n_acc - 1),
                            )
                            i += 1
                    ot = opool.tile([P, NF], fp32, tag="out_t")
                    nc.vector.tensor_add(
                        ot[:], ps[:], x_sb[:, half, co, :].bitcast(fp32))
                    nc.sync.dma_start(out_r[:, co, hw_sl], ot[:])

    nc.compile()
    return nc


def _get_nc():
    if MM_DTYPE not in _NC_CACHE:
        _NC_CACHE[MM_DTYPE] = _build_nc(MM_DTYPE)
    return _NC_CACHE[MM_DTYPE]


def _gate(inputs, k, Wg, bg):
    """Replicates the reference gate in fp32 numpy."""
    Bn = inputs.shape[0]
    pooled = inputs.mean(axis=(2, 3), dtype=np.float32)       # [B, C]
    logits = pooled.astype(np.float32) @ Wg.astype(np.float32) + bg  # [B, E]
    m = logits.max(axis=1, keepdims=True)
    ew = np.exp(logits - m)
    sm = ew / ew.sum(axis=1, keepdims=True)                   # [B, E] softmax
    idx = np.argsort(-sm, axis=1, kind="stable")[:, :E2]      # [B, 2]
    topw = np.take_along_axis(sm, idx, axis=1)                # [B, 2]
    s = (topw * k.reshape(Bn, 1)).astype(np.float32)          # [B, 2]
    return idx, s


def _pack_core_inputs(xb, W1sel, b1sel, W2s):
    """Pack one core's tensors into the per-partition SBUF layouts."""
    # x: [C, HW] -> [P, NH, CO, NF]  with x[co*P+p, hf*NF+f]
    xp = xb.reshape(CO, P, NH, NF).transpose(1, 2, 0, 3)
    # w1: [E2, C, DH] -> [P, E2, DO, CO, P]  w1[e, co*P+p, do*P+j]
    w1p = W1sel.reshape(E2, CO, P, DO, P).transpose(2, 0, 3, 1, 4)
    # b1: [E2, DH] -> [P, E2, DO]
    b1p = b1sel.reshape(E2, DO, P).transpose(2, 0, 1)
    # w2: [E2, DH, C] -> [P, E2, DO, C]
    w2p = W2s.reshape(E2, DO, P, C).transpose(2, 0, 1, 3)
    return {
        "x": np.ascontiguousarray(xp, dtype=np.float32),
        "w1": np.ascontiguousarray(w1p, dtype=np.float32),
        "b1": np.ascontiguousarray(b1p, dtype=np.float32),
        "w2": np.ascontiguousarray(w2p, dtype=np.float32),
    }


def kernel(inputs, k, Wg, bg, W1, b1, W2, b2):
    from concourse.bass_utils import run_bass_kernel_spmd

    inputs = np.asarray(inputs)
    Bn, Cn, Hn, Wn = inputs.shape
    idx, s = _gate(inputs, k, np.asarray(Wg), np.asarray(bg))

    x = np.ascontiguousarray(inputs.reshape(Bn, Cn, Hn * Wn)).astype(np.float32)
    W1 = np.asarray(W1, dtype=np.float32)
    b1 = np.asarray(b1, dtype=np.float32)
    W2 = np.asarray(W2, dtype=np.float32)
    b2 = np.asarray(b2, dtype=np.float32)

    in_maps = []
    for b in range(Bn):
        sel = idx[b]
        w2s = (W2[sel] * s[b, :, None, None]).astype(np.float32)
        in_maps.append(_pack_core_inputs(x[b], W1[sel], b1[sel], w2s))

    nc = _get_nc()
    res = run_bass_kernel_spmd(nc, in_maps, core_ids=list(range(Bn)))
    out = np.stack([res.results[b]["out"] for b in range(Bn)], axis=0)  # [B,C,HW]

    # b2 contribution: per-sample per-channel constant (zero in practice)
    bias_comb = np.einsum("bk,bkc->bc", s, b2[idx])           # [B, C]
    out = out + bias_comb[:, :, None]
    return out.reshape(Bn, Cn, Hn, Wn).astype(np.float32)


# revision 8
# speedup vs baseline: 1.1639x; 1.1639x over previous
"""MoE layer kernel for Trainium2 (8 NeuronCores, SPMD via bass/Tile).

Strategy:
  - Host: gate (global-avg-pool -> Linear -> softmax -> top-2). Only the
    top-2 experts per sample contribute to the output (exp_w is zero
    elsewhere), so we compute just those: 16 (sample, expert) pairs.
  - Device: core b processes sample b with its 2 selected experts.
    out = x + sum_e (s_e * W2_e)^T gelu(W1_e^T x + b1_e)
    where s_e = topk_w[b,e] * k[b] is folded into W2 on the host.
    The b2 contribution (sum_e s_e*b2_e, a per-channel constant) is added
    on the host afterwards (it is zero for this module's init anyway).
  - Matmuls run in float32r (fp32 data, 1 cycle/row on the PE at N=512).
  - All inputs are pre-packed on the host into the exact per-partition
    SBUF layout so every DMA is 128 large contiguous descriptors, and
    DMAs are split so compute starts as soon as the first tiles land.
"""

import os
import numpy as np

P = 128
C = 512
DH = 1024
HW = 1024
CO = C // P     # 4 chunks of C on partitions
DO = DH // P    # 8 chunks of Dh on partitions
NF = 512        # matmul moving-dim tile
NH = HW // NF   # 2
E2 = 2          # experts per sample (top-k)
B = 8

MM_DTYPE = os.environ.get("MOE_MM_DTYPE", "float32r")

_NC_CACHE = {}


def _build_nc(mm_dtype_name):
    import concourse.mybir as mybir
    import concourse.tile as tile
    from concourse import bacc

    fp32 = mybir.dt.float32
    mmdt = getattr(mybir.dt, mm_dtype_name)

    nc = bacc.Bacc("TRN2", target_bir_lowering=False, debug=False, num_devices=B)

    # DRAM inputs pre-packed to per-partition layout (host does the packing)
    x_d = nc.dram_tensor("x", [P, NH, CO, NF], mmdt, kind="ExternalInput")
    w1_d = nc.dram_tensor("w1", [P, E2, DO, CO, P], mmdt, kind="ExternalInput")
    b1_d = nc.dram_tensor("b1", [P, E2, DO], fp32, kind="ExternalInput")
    w2_d = nc.dram_tensor("w2", [P, E2, DO, C], mmdt, kind="ExternalInput")
    out_d = nc.dram_tensor("out", [C, HW], fp32, kind="ExternalOutput")

    with tile.TileContext(nc) as tc:
        with (
            tc.tile_pool(name="const", bufs=1) as cpool,
            tc.tile_pool(name="psh", bufs=4, space="PSUM") as ph_pool,
            tc.tile_pool(name="psy", bufs=4, space="PSUM") as py_pool,
            tc.tile_pool(name="outp", bufs=4) as opool,
        ):
            x_sb = cpool.tile([P, NH, CO, NF], mmdt)
            w1_sb = cpool.tile([P, E2, DO, CO, P], mmdt)
            b1_sb = cpool.tile([P, E2, DO], fp32)
            w2_sb = cpool.tile([P, E2, DO, C], mmdt)
            h_sb = cpool.tile([P, E2, DO, HW], mmdt)

            # DMAs in consumption order on the sync HWDGE ring (FIFO, so
            # transfers complete in need-order at full bandwidth). Exactly
            # 8 input DMAs -> no HWDGE-semaphore-lane stalls. b1 (tiny)
            # rides the scalar ring.
            nc.scalar.dma_start(b1_sb[:], b1_d.ap()[:])
            nc.sync.dma_start(x_sb[:, 0], x_d.ap()[:, 0])
            nc.sync.dma_start(w1_sb[:, 0, 0:4], w1_d.ap()[:, 0, 0:4])
            nc.sync.dma_start(w1_sb[:, 0, 4:8], w1_d.ap()[:, 0, 4:8])
            nc.sync.dma_start(w1_sb[:, 1], w1_d.ap()[:, 1])
            nc.sync.dma_start(x_sb[:, 1], x_d.ap()[:, 1])
            nc.sync.dma_start(w2_sb[:, 0], w2_d.ap()[:, 0])
            nc.sync.dma_start(w2_sb[:, 1], w2_d.ap()[:, 1])

            # Stage A: h[e] = gelu(W1_e^T x + b1_e)   (partitions: Dh chunk)
            for half in range(NH):
                hw_sl = slice(half * NF, (half + 1) * NF)
                for e in range(E2):
                    for do in range(DO):
                        ps = ph_pool.tile([P, NF], fp32, tag="ps_h")
                        for co in range(CO):
                            nc.tensor.matmul(
                                ps[:],
                                w1_sb[:, e, do, co, :],
                                x_sb[:, half, co, :],
                                start=(co == 0),
                                stop=(co == CO - 1),
                            )
                        nc.scalar.activation(
                            h_sb[:, e, do, hw_sl],
                            ps[:],
                            mybir.ActivationFunctionType.Gelu,
                            bias=b1_sb[:, e, do:do + 1],
                            scale=1.0,
                        )

            # Stage B: out = x + sum_e (s_e W2_e)^T h_e  (partitions: C chunk)
            out_r = out_d.ap().rearrange("(o p) f -> p o f", p=P)
            for half in range(NH):
                hw_sl = slice(half * NF, (half + 1) * NF)
                for co in range(CO):
                    ps = py_pool.tile([P, NF], fp32, tag="ps_y")
                    n_acc = E2 * DO
                    i = 0
                    for e in range(E2):
                        for do in range(DO):
                            nc.tensor.matmul(
                                ps[:],
                                w2_sb[:, e, do, co * P:(co + 1) * P],
                                h_sb[:, e, do, hw_sl],
                                start=(i == 0),
                                stop=(i == n_acc - 1),
                            )
                            i += 1
                    ot = opool.tile([P, NF], fp32, tag="out_t")
                    nc.vector.tensor_add(
                        ot[:], ps[:], x_sb[:, half, co, :].bitcast(fp32))
                    nc.scalar.dma_start(out_r[:, co, hw_sl], ot[:])

    nc.compile()
    return nc


def _get_nc():
    if MM_DTYPE not in _NC_CACHE:
        _NC_CACHE[MM_DTYPE] = _build_nc(MM_DTYPE)
    return _NC_CACHE[MM_DTYPE]


def _gate(inputs, k, Wg, bg):
    """Replicates the reference gate in fp32 numpy."""
    Bn = inputs.shape[0]
    pooled = inputs.mean(axis=(2, 3), dtype=np.float32)       # [B, C]
    logits = pooled.astype(np.float32) @ Wg.astype(np.float32) + bg  # [B, E]
    m = logits.max(axis=1, keepdims=True)
    ew = np.exp(logits - m)
    sm = ew / ew.sum(axis=1, keepdims=True)                   # [B, E] softmax
    idx = np.argsort(-sm, axis=1, kind="stable")[:, :E2]      # [B, 2]
    topw = np.take_along_axis(sm, idx, axis=1)                # [B, 2]
    s = (topw * k.reshape(Bn, 1)).astype(np.float32)          # [B, 2]
    return idx, s


def _pack_core_inputs(xb, W1sel, b1sel, W2s):
    """Pack one core's tensors into the per-partition SBUF layouts."""
    # x: [C, HW] -> [P, NH, CO, NF]  with x[co*P+p, hf*NF+f]
    xp = xb.reshape(CO, P, NH, NF).transpose(1, 2, 0, 3)
    # w1: [E2, C, DH] -> [P, E2, DO, CO, P]  w1[e, co*P+p, do*P+j]
    w1p = W1sel.reshape(E2, CO, P, DO, P).transpose(2, 0, 3, 1, 4)
    # b1: [E2, DH] -> [P, E2, DO]
    b1p = b1sel.reshape(E2, DO, P).transpose(2, 0, 1)
    # w2: [E2, DH, C] -> [P, E2, DO, C]
    w2p = W2s.reshape(E2, DO, P, C).transpose(2, 0, 1, 3)
    return {
        "x": np.ascontiguousarray(xp, dtype=np.float32),
        "w1": np.ascontiguousarray(w1p, dtype=np.float32),
        "b1": np.ascontiguousarray(b1p, dtype=np.float32),
        "w2": np.ascontiguousarray(w2p, dtype=np.float32),
    }


def kernel(inputs, k, Wg, bg, W1, b1, W2, b2):
    from concourse.bass_utils import run_bass_kernel_spmd

    inputs = np.asarray(inputs)
    Bn, Cn, Hn, Wn = inputs.shape
    idx, s = _gate(inputs, k, np.asarray(Wg), np.asarray(bg))

    x = np.ascontiguousarray(inputs.reshape(Bn, Cn, Hn * Wn)).astype(np.float32)
    W1 = np.asarray(W1, dtype=np.float32)
    b1 = np.asarray(b1, dtype=np.float32)
    W2 = np.asarray(W2, dtype=np.float32)
    b2 = np.asarray(b2, dtype=np.float32)

    in_maps = []
    for b in range(Bn):
        sel = idx[b]
        w2s = (W2[sel] * s[b, :, None, None]).astype(np.float32)
        in_maps.append(_pack_core_inputs(x[b], W1[sel], b1[sel], w2s))

    nc = _get_nc()
    res = run_bass_kernel_spmd(nc, in_maps, core_ids=list(range(Bn)))
    out = np.stack([res.results[b]["out"] for b in range(Bn)], axis=0)  # [B,C,HW]

    # b2 contribution: per-sample per-channel constant (zero in practice)
    bias_comb = np.einsum("bk,bkc->bc", s, b2[idx])           # [B, C]
    out = out + bias_comb[:, :, None]
    return out.reshape(Bn, Cn, Hn, Wn).astype(np.float32)
